# revision 13
# baseline (speedup 1.0000x reference)
"""Trainium2 Bass kernel for nn_PhysicsNetwork (gnn message passing).

Strategy
--------
Pure data parallel over 8 NeuronCores: each core gets N/8 = 8192 rows
(1024 scenes of K=8 entities); MLP weights replicated (packed into one
[128, W] tensor -> single DMA).

On-core dataflow: all activations live transposed [feature, row] in SBUF so
every MLP layer is a PE matmul  out[dout, rows] = W[din, dout].T @ x[din, rows]
with rows as the free dim (N=512 per matmul / one PSUM bank).

ELU is decomposed exactly (elu(z+b) = relu(z+b) + min(exp(z+b), 1) - 1, the
trailing "-1" folded into the *next* layer's bias, host-precomputed):
  1. e  = Exp(z + b)            ACT, bias via per-partition AP
  2. E  = min(e, 1) + b         GPSIMD/DVE tensor_scalar (two scalar stages)
  3. y1 = max(z, -b) + E        DVE scalar_tensor_tensor  (= elu(z+b) + 1)
Sigmoid = 0.5*tanh(0.5 x) + 0.5 with the affine folded into the PE broadcast
matmul, so only the exp/tanh ACT table set is ever loaded.

Attention gating (eff * sigmoid(att)) uses a PE rank-1 broadcast of the tanh
vector plus one DVE scalar_tensor_tensor: (eff1 - 1) * attB.  The per-entity
sum over the 7 partners is a PE matmul with a constant 0/1 block matrix.
"""

import numpy as np

K = 8
D = 128
DET = 64
STO = 64
A = 6
AE = 32
N_FULL = 65536
NCORES = 8
F = 512                       # matmul free dim / scene-chunk width
PAIRS = [(0, 1), (2, 3), (4, 5), (6,)]   # j-slot grouping into PSUM tiles
G1, G2 = 4, 3                 # j-slot groups for the interaction reduce

_BUILD_CACHE = {}


def _np32(x):
    return np.ascontiguousarray(np.asarray(x, dtype=np.float32))


def _prep_tensors(params):
    """Flatten params into named fp32 arrays [din, dout]; fold the elu(+1)
    corrections and sigmoid/broadcast constants (fixups in float64)."""
    def w(p):
        return np.asarray(p, dtype=np.float64)

    def adj(b, W):  # consume an elu1 (= elu+1) input: b' = b - sum_k W[k, :]
        return w(b) - w(W).sum(axis=0)

    p = params
    t = {}

    def put(name, arr, col=False):
        a = np.asarray(arr, dtype=np.float64).astype(np.float32)
        if col:
            a = a.reshape(-1, 1)
        t[name] = np.ascontiguousarray(a)

    put("wi1", p["inertia"]["w1"]); put("bi1", p["inertia"]["b1"], col=True)
    put("nbi1", -w(p["inertia"]["b1"]), col=True)
    put("wi2", p["inertia"]["w2"])
    bi2 = adj(p["inertia"]["b2"], p["inertia"]["w2"])
    put("bi2", bi2, col=True); put("nbi2", -bi2, col=True)

    put("wa1", p["action_enc"]["w1"]); put("ba1", p["action_enc"]["b1"], col=True)
    put("nba1", -w(p["action_enc"]["b1"]), col=True)
    put("wa2", p["action_enc"]["w2"])
    ba2 = adj(p["action_enc"]["b2"], p["action_enc"]["w2"])
    put("ba2", ba2, col=True); put("nba2", -ba2, col=True)

    we1 = w(p["action_effect"]["w1"])           # [160, 128]
    put("we1a", we1[:D]); put("we1b", we1[D:])
    be1 = adj(p["action_effect"]["b1"], we1)
    put("be1", be1, col=True); put("nbe1", -be1, col=True)
    put("we2", p["action_effect"]["w2"])
    be2 = adj(p["action_effect"]["b2"], p["action_effect"]["w2"])
    put("be2", be2, col=True); put("nbe2", -be2, col=True)

    wt1 = w(p["action_att"]["w1"])
    put("wt1a", wt1[:D]); put("wt1b", wt1[D:])
    bt1 = adj(p["action_att"]["b1"], wt1)
    put("bt1", bt1, col=True); put("nbt1", -bt1, col=True)
    put("wt2", p["action_att"]["w2"])           # [128, 1]
    bt2 = adj(p["action_att"]["b2"], p["action_att"]["w2"])
    put("tb1", 0.5 * bt2, col=True)

    wp1 = w(p["pairwise"]["w1"])                # [256, 256]
    put("wp1aa", wp1[:D, :D]);  put("wp1ba", wp1[D:, :D])
    put("wp1ab", wp1[:D, D:]);  put("wp1bb", wp1[D:, D:])
    bp1 = w(p["pairwise"]["b1"])
    put("bp1a", bp1[:D], col=True); put("nbp1a", -bp1[:D], col=True)
    put("bp1b", bp1[D:], col=True); put("nbp1b", -bp1[D:], col=True)
    wp2 = w(p["pairwise"]["w2"])                # [256, 128]
    put("wp2a", wp2[:D]); put("wp2b", wp2[D:])
    bp2 = adj(p["pairwise"]["b2"], wp2)
    put("bp2", bp2, col=True); put("nbp2", -bp2, col=True)

    put("wie1", p["int_effect"]["w1"])
    bie1 = adj(p["int_effect"]["b1"], p["int_effect"]["w1"])
    put("bie1", bie1, col=True); put("nbie1", -bie1, col=True)
    put("wie2", p["int_effect"]["w2"])          # [128, 32]
    bie2 = adj(p["int_effect"]["b2"], p["int_effect"]["w2"])   # [32]
    put("beff4", np.tile(bie2, G1), col=True)
    put("nbeff4", -np.tile(bie2, G1), col=True)
    put("beff3", np.tile(bie2, G2), col=True)
    put("nbeff3", -np.tile(bie2, G2), col=True)

    put("wia1", p["int_att"]["w1"])
    bia1 = adj(p["int_att"]["b1"], p["int_att"]["w1"])
    put("bia1", bia1, col=True); put("nbia1", -bia1, col=True)
    wia2 = w(p["int_att"]["w2"])                # [128, 1]
    bia2 = float(np.asarray(adj(p["int_att"]["b2"], wia2)).reshape(-1)[0])
    for cnt, tagn in ((G1, "4"), (G2, "3")):
        for sl in range(cnt):
            m = np.zeros((D, cnt))
            m[:, sl] = wia2[:, 0]
            put(f"wu{tagn}_{sl}", m)
        put(f"tb{tagn}", np.full((cnt, 1), 0.5 * bia2))

    wf1 = w(p["final"]["w1"])                   # [160, 128]
    put("wf1a", wf1[:D]); put("wf1b", wf1[D:])
    bf1 = w(p["final"]["b1"])
    put("bf1", bf1, col=True); put("nbf1", -bf1, col=True)
    put("wf2", p["final"]["w2"])
    bf2 = adj(p["final"]["b2"], p["final"]["w2"])
    put("bf2", bf2, col=True); put("nbf2", -bf2, col=True)

    for nm, key in (("d", "det_out"), ("l", "lambdas1"), ("m", "lambdas2")):
        put(f"w{nm}1", p[key]["w1"])
        b1h = adj(p[key]["b1"], p[key]["w1"])
        put(f"b{nm}1", b1h, col=True); put(f"nb{nm}1", -b1h, col=True)
        put(f"w{nm}2", p[key]["w2"])
        b2h = adj(p[key]["b2"], p[key]["w2"])
        put(f"b{nm}2", b2h, col=True)

    put("ident", np.eye(D))
    bd4 = np.zeros((G1, G1 * AE))
    for sl in range(G1):
        bd4[sl, sl * AE:(sl + 1) * AE] = 0.5
    put("bd4", bd4)
    bd3 = np.zeros((G2, G2 * AE))
    for sl in range(G2):
        bd3[sl, sl * AE:(sl + 1) * AE] = 0.5
    put("bd3", bd3)
    r4 = np.zeros((G1 * AE, AE))
    for sl in range(G1):
        r4[sl * AE:(sl + 1) * AE] = np.eye(AE)
    put("r4", r4)
    r3 = np.zeros((G2 * AE, AE))
    for sl in range(G2):
        r3[sl * AE:(sl + 1) * AE] = np.eye(AE)
    put("r3", r3)
    put("half", np.full((1, D), 0.5))
    put("onesrow", np.ones((1, F)))
    return t


def _pack_layout(tensors):
    """column layout for the single packed [128, W] weights tensor"""
    layout = {}
    col = 0
    for name in tensors:
        din, dout = tensors[name].shape
        assert din <= D
        layout[name] = (col, din, dout)
        col += dout
    return layout, col


def _pack(tensors, layout, tot):
    wp = np.zeros((D, tot), dtype=np.float32)
    for name, (c0, din, dout) in layout.items():
        wp[:din, c0:c0 + dout] = tensors[name]
    return wp


def _build_nc(rows, layout, tot, elu_mid_engine="gpsimd"):
    """Emit the Bass module for one core processing `rows` rows."""
    from contextlib import ExitStack

    import concourse.mybir as mybir
    import concourse.tile as tile
    from concourse import bacc

    fp32 = mybir.dt.float32
    AF = mybir.ActivationFunctionType
    OP = mybir.AluOpType

    scenes = rows // K
    n_rc = rows // F                 # stage A/C chunks
    n_sc = (scenes + F - 1) // F     # stage B chunks

    nc = bacc.Bacc()
    x_d = nc.dram_tensor("x", [rows, D], fp32, kind="ExternalInput")
    a_d = nc.dram_tensor("act", [rows, A], fp32, kind="ExternalInput")
    w_d = nc.dram_tensor("wpack", [D, tot], fp32, kind="ExternalInput")
    odet = nc.dram_tensor("odet", [rows, DET], fp32, kind="ExternalOutput")
    ol1 = nc.dram_tensor("ol1", [rows, STO], fp32, kind="ExternalOutput")
    ol2 = nc.dram_tensor("ol2", [rows, STO], fp32, kind="ExternalOutput")

    with tile.TileContext(nc) as tc, ExitStack() as ctx:
        wpool = ctx.enter_context(tc.tile_pool(name="weights", bufs=1))
        wtile = wpool.tile([D, tot], fp32, name="wtile")
        nc.sync.dma_start(out=wtile, in_=w_d[:])
        ws = {
            name: wtile[0:din, c0:c0 + dout]
            for name, (c0, din, dout) in layout.items()
        }

        persist = ctx.enter_context(tc.tile_pool(name="persist", bufs=1))
        senc = persist.tile([D, scenes, K], fp32, name="senc")      # gated state_enc
        totT = persist.tile([AE, scenes, K], fp32, name="totT")     # total_effect

        mid_eng = nc.gpsimd if elu_mid_engine == "gpsimd" else nc.vector

        def elu1(pool_e, z, b, nb, out_ap):
            """out = elu(z + b) + 1   (z in PSUM, out in SBUF)"""
            pdim = z.shape[0]
            fdim = int(np.prod(z.shape[1:]))
            e = pool_e.tile([pdim, fdim], fp32, name="e_t", tag="e_t")
            nc.scalar.activation(out=e, in_=z, func=AF.Exp, bias=b[:pdim], scale=1.0)
            E = pool_e.tile([pdim, fdim], fp32, name="E_t", tag="E_t")
            mid_eng.tensor_scalar(
                out=E, in0=e, scalar1=1.0, scalar2=b[:pdim],
                op0=OP.min, op1=OP.add,
            )
            nc.vector.scalar_tensor_tensor(
                out=out_ap, in0=z, scalar=nb[:pdim], in1=E,
                op0=OP.max, op1=OP.add,
            )

        # ---------------- stage A: per-row MLPs -> gated state_enc ----------
        with ExitStack() as actx:
            io = actx.enter_context(tc.tile_pool(name="a_io", bufs=3))
            sb = actx.enter_context(tc.tile_pool(name="a_sb", bufs=2))
            eb = actx.enter_context(tc.tile_pool(name="a_eb", bufs=3))
            pz = actx.enter_context(tc.tile_pool(name="a_pz", bufs=3, space="PSUM"))
            pt = actx.enter_context(tc.tile_pool(name="a_pt", bufs=1, space="PSUM"))
            psm = actx.enter_context(tc.tile_pool(name="a_psm", bufs=2, space="PSUM"))

            for c in range(n_rc):
                r0 = c * F
                nseg = F // D
                xs = io.tile([D, nseg, D], fp32, name="x_sb", tag="x_sb")
                nc.sync.dma_start(
                    out=xs,
                    in_=x_d[r0:r0 + F, :].rearrange("(t p) f -> p t f", p=D),
                )
                as_ = io.tile([D, nseg, A], fp32, name="a_sb", tag="a_sb")
                nc.sync.dma_start(
                    out=as_,
                    in_=a_d[r0:r0 + F, :].rearrange("(t p) f -> p t f", p=D),
                )
                xT_ps = pt.tile([D, F], fp32, name="xT_ps", tag="tp")
                aT_ps = pt.tile([A, F], fp32, name="aT_ps", tag="tp2")
                # PE fences: absorb the DMA-queue waits with regular matmuls
                # (transpose-mode matmuls have a single sync-wait slot).
                nc.tensor.matmul(xT_ps[0:1, 0:1], wtile[0:1, 0:1],
                                 xs[0:1, 0, 0:1], start=True, stop=True)
                nc.tensor.matmul(xT_ps[0:1, 1:2], wtile[0:1, 0:1],
                                 as_[0:1, 0, 0:1], start=True, stop=True)
                for t in range(nseg):
                    nc.tensor.transpose(
                        xT_ps[:, t * D:(t + 1) * D], xs[:, t, :], ws["ident"])
                    nc.tensor.transpose(
                        aT_ps[:, t * D:(t + 1) * D], as_[:, t, :], ws["ident"])
                xT = sb.tile([D, F], fp32, name="xT", tag="xT")
                nc.scalar.copy(out=xT, in_=xT_ps)
                aT = sb.tile([A, F], fp32, name="aT", tag="aT")
                nc.scalar.copy(out=aT, in_=aT_ps)

                def layer(parts, pdim, b, nb, outname):
                    z = pz.tile([pdim, F], fp32, name="a_z", tag="a_z")
                    for q, (lhsT, rhs) in enumerate(parts):
                        nc.tensor.matmul(
                            z, lhsT, rhs,
                            start=(q == 0), stop=(q == len(parts) - 1),
                        )
                    o = sb.tile([pdim, F], fp32, name=outname, tag=outname)
                    elu1(eb, z, ws[b], ws[nb], o)
                    return o

                hi = layer([(ws["wi1"], xT)], D, "bi1", "nbi1", "hi")
                s1 = layer([(ws["wi2"], hi)], D, "bi2", "nbi2", "s1")
                ha = layer([(ws["wa1"], aT)], D, "ba1", "nba1", "ha")
                ae_ = layer([(ws["wa2"], ha)], AE, "ba2", "nba2", "ae_")
                he = layer([(ws["we1a"], s1), (ws["we1b"], ae_)], D, "be1", "nbe1", "he")
                ef = layer([(ws["we2"], he)], D, "be2", "nbe2", "ef")
                ht = layer([(ws["wt1a"], s1), (ws["wt1b"], ae_)], D, "bt1", "nbt1", "ht")
                zt = psm.tile([1, F], fp32, name="a_zt", tag="a_sm")
                nc.tensor.matmul(zt, ws["wt2"], ht, start=True, stop=True)
                ts_ = sb.tile([1, F], fp32, name="ts_", tag="ts_")
                nc.scalar.activation(out=ts_, in_=zt, func=AF.Tanh,
                                     bias=ws["tb1"], scale=0.5)
                zb = psm.tile([D, F], fp32, name="a_zb", tag="a_sm")
                nc.tensor.matmul(zb, ws["half"], ts_, start=True, stop=False)
                nc.tensor.matmul(zb, ws["half"], ws["onesrow"][:, :F],
                                 start=False, stop=True)
                nc.vector.scalar_tensor_tensor(
                    out=senc[:, r0 // K:(r0 + F) // K, :],
                    in0=ef, scalar=1.0, in1=zb,
                    op0=OP.subtract, op1=OP.mult,
                )

        # ---------------- stage B: pairwise interactions --------------------
        with ExitStack() as bctx:
            hs = bctx.enter_context(tc.tile_pool(name="b_hs", bufs=1))
            hs2 = bctx.enter_context(tc.tile_pool(name="b_hs2", bufs=1))
            eb = bctx.enter_context(tc.tile_pool(name="b_eb", bufs=3))
            cs_p = bctx.enter_context(tc.tile_pool(name="b_cs", bufs=2))
            pbig = bctx.enter_context(tc.tile_pool(name="b_pbig", bufs=3, space="PSUM"))
            psml = bctx.enter_context(tc.tile_pool(name="b_psml", bufs=2, space="PSUM"))

            for csc in range(n_sc):
                s0 = csc * F
                scw = min(F, scenes - s0)
                for i in range(K):
                    jlist = [j for j in range(K) if j != i]
                    s_i = senc[:, s0:s0 + scw, i]

                    def pair_layer(mk_groups, pdim, b, nb, outname, pool):
                        out = pool.tile([pdim, 7 * scw], fp32,
                                        name=outname, tag=outname)
                        for js in PAIRS:
                            w = len(js) * scw
                            z = pbig.tile([pdim, w], fp32, name="b_z", tag="b_z")
                            groups = mk_groups(js)
                            for sl in range(len(js)):
                                for q, grp in enumerate(groups):
                                    lhsT, rhs = grp(sl)
                                    nc.tensor.matmul(
                                        z[:, sl * scw:(sl + 1) * scw], lhsT, rhs,
                                        start=(q == 0), stop=(q == len(groups) - 1),
                                    )
                            elu1(eb, z, ws[b], ws[nb],
                                 out[:, js[0] * scw:js[0] * scw + w])
                        return out

                    def l1_groups(wa, wb):
                        def mk(js):
                            return [
                                lambda sl: (ws[wa], s_i),
                                lambda sl, js=js: (
                                    ws[wb], senc[:, s0:s0 + scw, jlist[js[sl]]]),
                            ]
                        return mk
                    hA = pair_layer(l1_groups("wp1aa", "wp1ba"), D,
                                    "bp1a", "nbp1a", "hA", hs2)
                    hB = pair_layer(l1_groups("wp1ab", "wp1bb"), D,
                                    "bp1b", "nbp1b", "hB", hs2)

                    def l2_groups(js):
                        return [
                            lambda sl, js=js: (
                                ws["wp2a"], hA[:, (js[0] + sl) * scw:(js[0] + sl + 1) * scw]),
                            lambda sl, js=js: (
                                ws["wp2b"], hB[:, (js[0] + sl) * scw:(js[0] + sl + 1) * scw]),
                        ]
                    pw = pair_layer(l2_groups, D, "bp2", "nbp2", "pw", hs)

                    def i1_groups(wn):
                        def mk(js):
                            return [
                                lambda sl, js=js: (
                                    ws[wn], pw[:, (js[0] + sl) * scw:(js[0] + sl + 1) * scw]),
                            ]
                        return mk
                    hE = pair_layer(i1_groups("wie1"), D, "bie1", "nbie1", "hE", hs)
                    hT = pair_layer(i1_groups("wia1"), D, "bia1", "nbia1", "hT", hs)

                    cgs = []
                    for g, (j0, cnt) in enumerate([(0, G1), (G1, G2)]):
                        tg = "4" if cnt == G1 else "3"
                        zF = psml.tile([cnt * AE, scw], fp32, name="b_zF", tag="b_sm")
                        for sl in range(cnt):
                            nc.tensor.matmul(
                                zF[sl * AE:(sl + 1) * AE, :], ws["wie2"],
                                hE[:, (j0 + sl) * scw:(j0 + sl + 1) * scw],
                                start=True, stop=True,
                                tile_position=(0, sl * AE),
                            )
                        eg = cs_p.tile([cnt * AE, scw], fp32, name="eg", tag="eg")
                        elu1(eb, zF, ws[f"beff{tg}"], ws[f"nbeff{tg}"], eg)

                        zU = psml.tile([cnt, scw], fp32, name="b_zU", tag="b_sm")
                        for sl in range(cnt):
                            nc.tensor.matmul(
                                zU, ws[f"wu{tg}_{sl}"],
                                hT[:, (j0 + sl) * scw:(j0 + sl + 1) * scw],
                                start=(sl == 0), stop=(sl == cnt - 1),
                            )
                        tv = cs_p.tile([cnt, scw], fp32, name="tv", tag="tv")
                        nc.scalar.activation(out=tv, in_=zU, func=AF.Tanh,
                                             bias=ws[f"tb{tg}"], scale=0.5)
                        zB = psml.tile([cnt * AE, scw], fp32, name="b_zB", tag="b_sm")
                        nc.tensor.matmul(zB, ws[f"bd{tg}"], tv, start=True, stop=False)
                        nc.tensor.matmul(zB, ws["half"][:, :cnt * AE],
                                         ws["onesrow"][:, :scw], start=False, stop=True)
                        cg = cs_p.tile([cnt * AE, scw], fp32, name="cg", tag="cg")
                        nc.vector.scalar_tensor_tensor(
                            out=cg, in0=eg, scalar=1.0, in1=zB,
                            op0=OP.subtract, op1=OP.mult,
                        )
                        cgs.append(cg)

                    zR = psml.tile([AE, scw], fp32, name="b_zR", tag="b_sm")
                    nc.tensor.matmul(zR, ws["r4"], cgs[0], start=True, stop=False)
                    nc.tensor.matmul(zR, ws["r3"], cgs[1], start=False, stop=True)
                    nc.scalar.copy(out=totT[:, s0:s0 + scw, i], in_=zR)

        # ---------------- stage C: final MLP + heads + output transpose -----
        with ExitStack() as cctx:
            sb = cctx.enter_context(tc.tile_pool(name="c_sb", bufs=2))
            eb = cctx.enter_context(tc.tile_pool(name="c_eb", bufs=3))
            ob = cctx.enter_context(tc.tile_pool(name="c_ob", bufs=3))
            pz = cctx.enter_context(tc.tile_pool(name="c_pz", bufs=3, space="PSUM"))
            po = cctx.enter_context(tc.tile_pool(name="c_po", bufs=2, space="PSUM"))

            for c in range(n_rc):
                r0 = c * F
                srhs = senc[:, r0 // K:(r0 + F) // K, :]
                trhs = totT[:, r0 // K:(r0 + F) // K, :]

                def layer(parts, pdim, b, nb, outname):
                    z = pz.tile([pdim, F], fp32, name="c_z", tag="c_z")
                    for q, (lhsT, rhs) in enumerate(parts):
                        nc.tensor.matmul(z, lhsT, rhs,
                                         start=(q == 0), stop=(q == len(parts) - 1))
                    o = sb.tile([pdim, F], fp32, name=outname, tag=outname)
                    elu1(eb, z, ws[b], ws[nb], o)
                    return o

                hf = layer([(ws["wf1a"], srhs), (ws["wf1b"], trhs)], D,
                           "bf1", "nbf1", "hf")
                ag = layer([(ws["wf2"], hf)], D, "bf2", "nbf2", "ag")

                outs = []
                for nm in ("d", "l", "m"):
                    hh = layer([(ws[f"w{nm}1"], ag)], D, f"b{nm}1", f"nb{nm}1",
                               f"h{nm}")
                    zo = pz.tile([DET, F], fp32, name="c_zo", tag="c_z")
                    nc.tensor.matmul(zo, ws[f"w{nm}2"], hh, start=True, stop=True)
                    oo = sb.tile([DET, F], fp32, name=f"o{nm}", tag=f"o{nm}")
                    nc.scalar.activation(out=oo, in_=zo, func=AF.Identity,
                                         bias=ws[f"b{nm}2"], scale=1.0)
                    outs.append(oo)

                for t in range(F // D):
                    pot = po.tile([D, 3 * DET], fp32, name="pot", tag="pot")
                    for q, oo in enumerate(outs):
                        nc.tensor.transpose(
                            pot[:, q * DET:(q + 1) * DET],
                            oo[:, t * D:(t + 1) * D],
                            ws["ident"][:DET, :DET],
                        )
                    st = ob.tile([D, 3 * DET], fp32, name="st", tag="st")
                    nc.scalar.copy(out=st, in_=pot)
                    ra, rb = r0 + t * D, r0 + (t + 1) * D
                    nc.sync.dma_start(out=odet[ra:rb, :], in_=st[:, 0:DET])
                    nc.sync.dma_start(out=ol1[ra:rb, :], in_=st[:, DET:2 * DET])
                    nc.sync.dma_start(out=ol2[ra:rb, :], in_=st[:, 2 * DET:3 * DET])

    nc.finalize()
    return nc


def _get_nc(rows, layout, tot, elu_mid_engine="gpsimd"):
    key = (rows, elu_mid_engine)
    if key not in _BUILD_CACHE:
        _BUILD_CACHE[key] = _build_nc(rows, layout, tot, elu_mid_engine)
    return _BUILD_CACHE[key]


def kernel(sampled_state, actions, params, _trace=False, _ncores=NCORES):
    from concourse.bass_utils import run_bass_kernel_spmd

    x = _np32(sampled_state)
    a = _np32(actions)
    n = x.shape[0]
    rows = n // _ncores
    assert rows % F == 0

    tensors = _prep_tensors(params)
    layout, tot = _pack_layout(tensors)
    wpack = _pack(tensors, layout, tot)
    nc = _get_nc(rows, layout, tot)

    in_maps = []
    for c in range(_ncores):
        in_maps.append({
            "x": np.ascontiguousarray(x[c * rows:(c + 1) * rows]),
            "act": np.ascontiguousarray(a[c * rows:(c + 1) * rows]),
            "wpack": wpack,
        })

    res = run_bass_kernel_spmd(
        nc, in_maps, core_ids=list(range(_ncores)), trace=_trace,
    )
    results = res.results
    det = np.concatenate([r["odet"] for r in results], axis=0)
    l1 = np.concatenate([r["ol1"] for r in results], axis=0)
    l2 = np.concatenate([r["ol2"] for r in results], axis=0)
    if _trace:
        return (det, l1, l2), res
    return (det, l1, l2)


# revision 19
# speedup vs baseline: 5.1401x; 5.1401x over previous
"""Trainium2 Bass kernel for nn_PhysicsNetwork (gnn message passing).

Strategy
--------
Pure data parallel over 8 NeuronCores: each core gets N/8 = 8192 rows
(1024 scenes of K=8 entities); MLP weights replicated (packed into one
[128, W] tensor -> single DMA).

On-core dataflow: all activations live transposed [feature, row] in SBUF so
every MLP layer is a PE matmul  out[dout, rows] = W[din, dout].T @ x[din, rows]
with rows as the free dim (N=512 per matmul / one PSUM bank).

ELU is decomposed exactly (elu(z+b) = relu(z+b) + min(exp(z+b), 1) - 1, the
trailing "-1" folded into the *next* layer's bias, host-precomputed):
  1. e  = Exp(z + b)            ACT, bias via per-partition AP
  2. E  = min(e, 1) + b         GPSIMD/DVE tensor_scalar (two scalar stages)
  3. y1 = max(z, -b) + E        DVE scalar_tensor_tensor  (= elu(z+b) + 1)
Sigmoid = 0.5*tanh(0.5 x) + 0.5 with the affine folded into the PE broadcast
matmul, so only the exp/tanh ACT table set is ever loaded.

Attention gating (eff * sigmoid(att)) uses a PE rank-1 broadcast of the tanh
vector plus one DVE scalar_tensor_tensor: (eff1 - 1) * attB.  The per-entity
sum over the 7 partners is a PE matmul with a constant 0/1 block matrix.
"""

import numpy as np

K = 8
D = 128
DET = 64
STO = 64
A = 6
AE = 32
N_FULL = 65536
NCORES = 8
F = 512                       # matmul free dim / scene-chunk width
PAIRS = [(0, 1), (2, 3), (4, 5), (6,)]   # j-slot grouping into PSUM tiles
G1, G2 = 4, 3                 # j-slot groups for the interaction reduce

_BUILD_CACHE = {}


def _np32(x):
    return np.ascontiguousarray(np.asarray(x, dtype=np.float32))


def _prep_tensors(params):
    """Flatten params into named fp32 arrays [din, dout]; fold the elu(+1)
    corrections and sigmoid/broadcast constants (fixups in float64)."""
    def w(p):
        return np.asarray(p, dtype=np.float64)

    def adj(b, W):  # consume an elu1 (= elu+1) input: b' = b - sum_k W[k, :]
        return w(b) - w(W).sum(axis=0)

    p = params
    t = {}

    def put(name, arr, col=False):
        a = np.asarray(arr, dtype=np.float64).astype(np.float32)
        if col:
            a = a.reshape(-1, 1)
        t[name] = np.ascontiguousarray(a)

    put("wi1", p["inertia"]["w1"]); put("bi1", p["inertia"]["b1"], col=True)
    put("nbi1", -w(p["inertia"]["b1"]), col=True)
    put("wi2", p["inertia"]["w2"])
    bi2 = adj(p["inertia"]["b2"], p["inertia"]["w2"])
    put("bi2", bi2, col=True); put("nbi2", -bi2, col=True)

    put("wa1", p["action_enc"]["w1"]); put("ba1", p["action_enc"]["b1"], col=True)
    put("nba1", -w(p["action_enc"]["b1"]), col=True)
    put("wa2", p["action_enc"]["w2"])
    ba2 = adj(p["action_enc"]["b2"], p["action_enc"]["w2"])
    put("ba2", ba2, col=True); put("nba2", -ba2, col=True)

    we1 = w(p["action_effect"]["w1"])           # [160, 128]
    put("we1a", we1[:D]); put("we1b", we1[D:])
    be1 = adj(p["action_effect"]["b1"], we1)
    put("be1", be1, col=True); put("nbe1", -be1, col=True)
    put("we2", p["action_effect"]["w2"])
    be2 = adj(p["action_effect"]["b2"], p["action_effect"]["w2"])
    put("be2", be2, col=True); put("nbe2", -be2, col=True)

    wt1 = w(p["action_att"]["w1"])
    put("wt1a", wt1[:D]); put("wt1b", wt1[D:])
    bt1 = adj(p["action_att"]["b1"], wt1)
    put("bt1", bt1, col=True); put("nbt1", -bt1, col=True)
    put("wt2", p["action_att"]["w2"])           # [128, 1]
    bt2 = adj(p["action_att"]["b2"], p["action_att"]["w2"])
    put("tb1", 0.5 * bt2, col=True)

    wp1 = w(p["pairwise"]["w1"])                # [256, 256]
    put("wp1aa", wp1[:D, :D]);  put("wp1ba", wp1[D:, :D])
    put("wp1ab", wp1[:D, D:]);  put("wp1bb", wp1[D:, D:])
    bp1 = w(p["pairwise"]["b1"])
    put("bp1a", bp1[:D], col=True); put("nbp1a", -bp1[:D], col=True)
    put("bp1b", bp1[D:], col=True); put("nbp1b", -bp1[D:], col=True)
    wp2 = w(p["pairwise"]["w2"])                # [256, 128]
    put("wp2a", wp2[:D]); put("wp2b", wp2[D:])
    bp2 = adj(p["pairwise"]["b2"], wp2)
    put("bp2", bp2, col=True); put("nbp2", -bp2, col=True)

    put("wie1", p["int_effect"]["w1"])
    bie1 = adj(p["int_effect"]["b1"], p["int_effect"]["w1"])
    put("bie1", bie1, col=True); put("nbie1", -bie1, col=True)
    put("wie2", p["int_effect"]["w2"])          # [128, 32]
    bie2 = adj(p["int_effect"]["b2"], p["int_effect"]["w2"])   # [32]
    put("beff4", np.tile(bie2, G1), col=True)
    put("nbeff4", -np.tile(bie2, G1), col=True)
    put("beff3", np.tile(bie2, G2), col=True)
    put("nbeff3", -np.tile(bie2, G2), col=True)

    put("wia1", p["int_att"]["w1"])
    bia1 = adj(p["int_att"]["b1"], p["int_att"]["w1"])
    put("bia1", bia1, col=True); put("nbia1", -bia1, col=True)
    wia2 = w(p["int_att"]["w2"])                # [128, 1]
    bia2 = float(np.asarray(adj(p["int_att"]["b2"], wia2)).reshape(-1)[0])
    for cnt, tagn in ((G1, "4"), (G2, "3")):
        for sl in range(cnt):
            m = np.zeros((D, cnt))
            m[:, sl] = wia2[:, 0]
            put(f"wu{tagn}_{sl}", m)
        put(f"tb{tagn}", np.full((cnt, 1), 0.5 * bia2))

    wf1 = w(p["final"]["w1"])                   # [160, 128]
    put("wf1a", wf1[:D]); put("wf1b", wf1[D:])
    bf1 = w(p["final"]["b1"])
    put("bf1", bf1, col=True); put("nbf1", -bf1, col=True)
    put("wf2", p["final"]["w2"])
    bf2 = adj(p["final"]["b2"], p["final"]["w2"])
    put("bf2", bf2, col=True); put("nbf2", -bf2, col=True)

    for nm, key in (("d", "det_out"), ("l", "lambdas1"), ("m", "lambdas2")):
        put(f"w{nm}1", p[key]["w1"])
        b1h = adj(p[key]["b1"], p[key]["w1"])
        put(f"b{nm}1", b1h, col=True); put(f"nb{nm}1", -b1h, col=True)
        put(f"w{nm}2", p[key]["w2"])
        b2h = adj(p[key]["b2"], p[key]["w2"])
        put(f"b{nm}2", b2h, col=True)

    put("ident", np.eye(D))
    bd4 = np.zeros((G1, G1 * AE))
    for sl in range(G1):
        bd4[sl, sl * AE:(sl + 1) * AE] = 0.5
    put("bd4", bd4)
    bd3 = np.zeros((G2, G2 * AE))
    for sl in range(G2):
        bd3[sl, sl * AE:(sl + 1) * AE] = 0.5
    put("bd3", bd3)
    r4 = np.zeros((G1 * AE, AE))
    for sl in range(G1):
        r4[sl * AE:(sl + 1) * AE] = np.eye(AE)
    put("r4", r4)
    r3 = np.zeros((G2 * AE, AE))
    for sl in range(G2):
        r3[sl * AE:(sl + 1) * AE] = np.eye(AE)
    put("r3", r3)
    put("half", np.full((1, D), 0.5))
    put("onesrow", np.ones((1, F)))
    return t


def _pack_layout(tensors):
    """column layout for the single packed [128, W] weights tensor"""
    layout = {}
    col = 0
    for name in tensors:
        din, dout = tensors[name].shape
        assert din <= D
        layout[name] = (col, din, dout)
        col += dout
    return layout, col


def _pack(tensors, layout, tot):
    wp = np.zeros((D, tot), dtype=np.float32)
    for name, (c0, din, dout) in layout.items():
        wp[:din, c0:c0 + dout] = tensors[name]
    return wp


def _build_nc(rows, layout, tot, elu_mid_engine="vector"):
    """Emit the Bass module for one core processing `rows` rows."""
    from contextlib import ExitStack

    import concourse.mybir as mybir
    import concourse.tile as tile
    from concourse import bacc

    fp32 = mybir.dt.float32
    r32 = mybir.dt.float32r
    AF = mybir.ActivationFunctionType
    OP = mybir.AluOpType

    scenes = rows // K
    n_rc = rows // F                 # stage A/C chunks
    n_sc = (scenes + F - 1) // F     # stage B chunks

    nc = bacc.Bacc()
    x_d = nc.dram_tensor("x", [rows, D], fp32, kind="ExternalInput")
    a_d = nc.dram_tensor("act", [rows, A], fp32, kind="ExternalInput")
    w_d = nc.dram_tensor("wpack", [D, tot], mybir.dt.float32r, kind="ExternalInput")
    odet = nc.dram_tensor("odet", [rows, DET], fp32, kind="ExternalOutput")
    ol1 = nc.dram_tensor("ol1", [rows, STO], fp32, kind="ExternalOutput")
    ol2 = nc.dram_tensor("ol2", [rows, STO], fp32, kind="ExternalOutput")

    with tile.TileContext(nc) as tc, ExitStack() as ctx:
        wpool = ctx.enter_context(tc.tile_pool(name="weights", bufs=1))
        wtile = wpool.tile([D, tot], mybir.dt.float32r, name="wtile")
        nc.sync.dma_start(out=wtile, in_=w_d[:])
        ws = {
            name: wtile[0:din, c0:c0 + dout]
            for name, (c0, din, dout) in layout.items()
        }

        persist = ctx.enter_context(tc.tile_pool(name="persist", bufs=1))
        senc = persist.tile([D, scenes, K], r32, name="senc")      # gated state_enc
        totT = persist.tile([AE, scenes, K], r32, name="totT")     # total_effect

        mid_eng = nc.gpsimd if elu_mid_engine == "gpsimd" else nc.vector

        def mm(out, lhsT, rhs, **kw):
            # float32r operands: single-pass PE matmul (4x over fp32)
            nc.tensor.matmul(out, lhsT, rhs, **kw)

        def fb(ap):
            # fp32 view of a float32r weight slice (bias/scalar operands)
            return ap.bitcast(fp32)

        def mmf(out, lhsT, rhs, **kw):
            # fp32 2-pass matmul for dst partitions FP32R can't encode
            nc.tensor.matmul(out, fb(lhsT), fb(rhs), **kw)

        def elu1(pool_e, z, b, nb, out_ap):
            """out = elu(z + b) + 1   (z in PSUM, out in SBUF)"""
            pdim = z.shape[0]
            fdim = int(np.prod(z.shape[1:]))
            e = pool_e.tile([pdim, fdim], fp32, name="e_t", tag="e_t")
            nc.scalar.activation(out=e, in_=z, func=AF.Exp, bias=fb(b[:pdim]), scale=1.0)
            E = pool_e.tile([pdim, fdim], fp32, name="E_t", tag="E_t")
            mid_eng.tensor_scalar(
                out=E, in0=e, scalar1=1.0, scalar2=fb(b[:pdim]),
                op0=OP.min, op1=OP.add,
            )
            nc.vector.scalar_tensor_tensor(
                out=out_ap, in0=z, scalar=fb(nb[:pdim]), in1=E,
                op0=OP.max, op1=OP.add,
            )

        # ---------------- stage A: per-row MLPs -> gated state_enc ----------
        with ExitStack() as actx:
            io = actx.enter_context(tc.tile_pool(name="a_io", bufs=3))
            sb = actx.enter_context(tc.tile_pool(name="a_sb", bufs=2))
            eb = actx.enter_context(tc.tile_pool(name="a_eb", bufs=3))
            pz = actx.enter_context(tc.tile_pool(name="a_pz", bufs=3, space="PSUM"))
            pt = actx.enter_context(tc.tile_pool(name="a_pt", bufs=1, space="PSUM"))
            psm = actx.enter_context(tc.tile_pool(name="a_psm", bufs=2, space="PSUM"))

            for c in range(n_rc):
                r0 = c * F
                nseg = F // D
                xs = io.tile([D, nseg, D], fp32, name="x_sb", tag="x_sb")
                nc.sync.dma_start(
                    out=xs,
                    in_=x_d[r0:r0 + F, :].rearrange("(t p) f -> p t f", p=D),
                )
                as_ = io.tile([D, nseg, A], fp32, name="a_sb", tag="a_sb")
                nc.sync.dma_start(
                    out=as_,
                    in_=a_d[r0:r0 + F, :].rearrange("(t p) f -> p t f", p=D),
                )
                xT_ps = pt.tile([D, F], fp32, name="xT_ps", tag="tp")
                aT_ps = pt.tile([A, F], fp32, name="aT_ps", tag="tp2")
                # PE fences: absorb the DMA-queue waits with regular matmuls
                # (transpose-mode matmuls have a single sync-wait slot).
                nc.tensor.matmul(xT_ps[0:1, 0:1], wtile.bitcast(fp32)[0:1, 0:1],
                                 xs[0:1, 0, 0:1], start=True, stop=True)
                nc.tensor.matmul(xT_ps[0:1, 1:2], wtile.bitcast(fp32)[0:1, 0:1],
                                 as_[0:1, 0, 0:1], start=True, stop=True)
                for t in range(nseg):
                    nc.tensor.transpose(
                        xT_ps[:, t * D:(t + 1) * D], xs[:, t, :], ws["ident"].bitcast(fp32))
                    nc.tensor.transpose(
                        aT_ps[:, t * D:(t + 1) * D], as_[:, t, :], ws["ident"].bitcast(fp32))
                xT = sb.tile([D, F], r32, name="xT", tag="xT")
                nc.scalar.copy(out=xT, in_=xT_ps)
                aT = sb.tile([A, F], r32, name="aT", tag="aT")
                nc.scalar.copy(out=aT, in_=aT_ps)

                def layer(parts, pdim, b, nb, outname):
                    z = pz.tile([pdim, F], fp32, name="a_z", tag="a_z")
                    for q, (lhsT, rhs) in enumerate(parts):
                        mm(
                            z, lhsT, rhs,
                            start=(q == 0), stop=(q == len(parts) - 1),
                        )
                    o = sb.tile([pdim, F], r32, name=outname, tag=outname)
                    elu1(eb, z, ws[b], ws[nb], o)
                    return o

                hi = layer([(ws["wi1"], xT)], D, "bi1", "nbi1", "hi")
                s1 = layer([(ws["wi2"], hi)], D, "bi2", "nbi2", "s1")
                ha = layer([(ws["wa1"], aT)], D, "ba1", "nba1", "ha")
                ae_ = layer([(ws["wa2"], ha)], AE, "ba2", "nba2", "ae_")
                he = layer([(ws["we1a"], s1), (ws["we1b"], ae_)], D, "be1", "nbe1", "he")
                ef = layer([(ws["we2"], he)], D, "be2", "nbe2", "ef")
                ht = layer([(ws["wt1a"], s1), (ws["wt1b"], ae_)], D, "bt1", "nbt1", "ht")
                zt = psm.tile([1, F], fp32, name="a_zt", tag="a_sm")
                mm(zt, ws["wt2"], ht, start=True, stop=True)
                ts_ = sb.tile([1, F], r32, name="ts_", tag="ts_")
                nc.scalar.activation(out=ts_, in_=zt, func=AF.Tanh,
                                     bias=fb(ws["tb1"]), scale=0.5)
                zb = psm.tile([D, F], fp32, name="a_zb", tag="a_sm")
                mm(zb, ws["half"], ts_, start=True, stop=False)
                mm(zb, ws["half"], ws["onesrow"][:, :F],
                                 start=False, stop=True)
                nc.vector.scalar_tensor_tensor(
                    out=senc[:, r0 // K:(r0 + F) // K, :],
                    in0=ef, scalar=1.0, in1=zb,
                    op0=OP.subtract, op1=OP.mult,
                )

        # ---------------- stage B: pairwise interactions --------------------
        with ExitStack() as bctx:
            hs = bctx.enter_context(tc.tile_pool(name="b_hs", bufs=1))
            hs2 = bctx.enter_context(tc.tile_pool(name="b_hs2", bufs=1))
            eb = bctx.enter_context(tc.tile_pool(name="b_eb", bufs=3))
            cs_p = bctx.enter_context(tc.tile_pool(name="b_cs", bufs=2))
            pbig = bctx.enter_context(tc.tile_pool(name="b_pbig", bufs=3, space="PSUM"))
            psml = bctx.enter_context(tc.tile_pool(name="b_psml", bufs=2, space="PSUM"))

            for csc in range(n_sc):
                s0 = csc * F
                scw = min(F, scenes - s0)
                for i in range(K):
                    jlist = [j for j in range(K) if j != i]
                    s_i = senc[:, s0:s0 + scw, i]

                    def pair_layer(mk_groups, pdim, b, nb, outname, pool):
                        out = pool.tile([pdim, 7 * scw], r32,
                                        name=outname, tag=outname)
                        for js in PAIRS:
                            w = len(js) * scw
                            z = pbig.tile([pdim, w], fp32, name="b_z", tag="b_z")
                            groups = mk_groups(js)
                            for sl in range(len(js)):
                                for q, grp in enumerate(groups):
                                    lhsT, rhs = grp(sl)
                                    mm(
                                        z[:, sl * scw:(sl + 1) * scw], lhsT, rhs,
                                        start=(q == 0), stop=(q == len(groups) - 1),
                                    )
                            elu1(eb, z, ws[b], ws[nb],
                                 out[:, js[0] * scw:js[0] * scw + w])
                        return out

                    def l1_groups(wa, wb):
                        def mk(js):
                            return [
                                lambda sl: (ws[wa], s_i),
                                lambda sl, js=js: (
                                    ws[wb], senc[:, s0:s0 + scw, jlist[js[sl]]]),
                            ]
                        return mk
                    hA = pair_layer(l1_groups("wp1aa", "wp1ba"), D,
                                    "bp1a", "nbp1a", "hA", hs2)
                    hB = pair_layer(l1_groups("wp1ab", "wp1bb"), D,
                                    "bp1b", "nbp1b", "hB", hs2)

                    def l2_groups(js):
                        return [
                            lambda sl, js=js: (
                                ws["wp2a"], hA[:, (js[0] + sl) * scw:(js[0] + sl + 1) * scw]),
                            lambda sl, js=js: (
                                ws["wp2b"], hB[:, (js[0] + sl) * scw:(js[0] + sl + 1) * scw]),
                        ]
                    pw = pair_layer(l2_groups, D, "bp2", "nbp2", "pw", hs)

                    def i1_groups(wn):
                        def mk(js):
                            return [
                                lambda sl, js=js: (
                                    ws[wn], pw[:, (js[0] + sl) * scw:(js[0] + sl + 1) * scw]),
                            ]
                        return mk
                    hE = pair_layer(i1_groups("wie1"), D, "bie1", "nbie1", "hE", hs)
                    hT = pair_layer(i1_groups("wia1"), D, "bia1", "nbia1", "hT", hs)

                    cgs = []
                    for g, (j0, cnt) in enumerate([(0, G1), (G1, G2)]):
                        tg = "4" if cnt == G1 else "3"
                        zF = psml.tile([cnt * AE, scw], fp32, name="b_zF", tag="b_sm")
                        for sl in range(cnt):
                            mmf(
                                zF[sl * AE:(sl + 1) * AE, :], ws["wie2"],
                                hE[:, (j0 + sl) * scw:(j0 + sl + 1) * scw],
                                start=True, stop=True,
                                tile_position=(0, sl * AE),
                            )
                        eg = cs_p.tile([cnt * AE, scw], r32, name="eg", tag="eg")
                        elu1(eb, zF, ws[f"beff{tg}"], ws[f"nbeff{tg}"], eg)

                        zU = psml.tile([cnt, scw], fp32, name="b_zU", tag="b_sm")
                        for sl in range(cnt):
                            mm(
                                zU, ws[f"wu{tg}_{sl}"],
                                hT[:, (j0 + sl) * scw:(j0 + sl + 1) * scw],
                                start=(sl == 0), stop=(sl == cnt - 1),
                            )
                        tv = cs_p.tile([cnt, scw], r32, name="tv", tag="tv")
                        nc.scalar.activation(out=tv, in_=zU, func=AF.Tanh,
                                             bias=fb(ws[f"tb{tg}"]), scale=0.5)
                        zB = psml.tile([cnt * AE, scw], fp32, name="b_zB", tag="b_sm")
                        mm(zB, ws[f"bd{tg}"], tv, start=True, stop=False)
                        mm(zB, ws["half"][:, :cnt * AE],
                                         ws["onesrow"][:, :scw], start=False, stop=True)
                        cg = cs_p.tile([cnt * AE, scw], r32, name="cg", tag="cg")
                        nc.vector.scalar_tensor_tensor(
                            out=cg, in0=eg, scalar=1.0, in1=zB,
                            op0=OP.subtract, op1=OP.mult,
                        )
                        cgs.append(cg)

                    zR = psml.tile([AE, scw], fp32, name="b_zR", tag="b_sm")
                    mm(zR, ws["r4"], cgs[0], start=True, stop=False)
                    mm(zR, ws["r3"], cgs[1], start=False, stop=True)
                    nc.scalar.copy(out=totT[:, s0:s0 + scw, i], in_=zR)

        # ---------------- stage C: final MLP + heads + output transpose -----
        with ExitStack() as cctx:
            sb = cctx.enter_context(tc.tile_pool(name="c_sb", bufs=2))
            eb = cctx.enter_context(tc.tile_pool(name="c_eb", bufs=3))
            ob = cctx.enter_context(tc.tile_pool(name="c_ob", bufs=3))
            pz = cctx.enter_context(tc.tile_pool(name="c_pz", bufs=3, space="PSUM"))
            po = cctx.enter_context(tc.tile_pool(name="c_po", bufs=2, space="PSUM"))

            for c in range(n_rc):
                r0 = c * F
                srhs = senc[:, r0 // K:(r0 + F) // K, :]
                trhs = totT[:, r0 // K:(r0 + F) // K, :]

                def layer(parts, pdim, b, nb, outname):
                    z = pz.tile([pdim, F], fp32, name="c_z", tag="c_z")
                    for q, (lhsT, rhs) in enumerate(parts):
                        mm(z, lhsT, rhs,
                                         start=(q == 0), stop=(q == len(parts) - 1))
                    o = sb.tile([pdim, F], r32, name=outname, tag=outname)
                    elu1(eb, z, ws[b], ws[nb], o)
                    return o

                hf = layer([(ws["wf1a"], srhs), (ws["wf1b"], trhs)], D,
                           "bf1", "nbf1", "hf")
                ag = layer([(ws["wf2"], hf)], D, "bf2", "nbf2", "ag")

                outs = []
                for nm in ("d", "l", "m"):
                    hh = layer([(ws[f"w{nm}1"], ag)], D, f"b{nm}1", f"nb{nm}1",
                               f"h{nm}")
                    zo = pz.tile([DET, F], fp32, name="c_zo", tag="c_z")
                    mm(zo, ws[f"w{nm}2"], hh, start=True, stop=True)
                    oo = sb.tile([DET, F], fp32, name=f"o{nm}", tag=f"o{nm}")
                    nc.scalar.activation(out=oo, in_=zo, func=AF.Identity,
                                         bias=fb(ws[f"b{nm}2"]), scale=1.0)
                    outs.append(oo)

                for t in range(F // D):
                    pot = po.tile([D, 3 * DET], fp32, name="pot", tag="pot")
                    for q, oo in enumerate(outs):
                        nc.tensor.transpose(
                            pot[:, q * DET:(q + 1) * DET],
                            oo[:, t * D:(t + 1) * D],
                            ws["ident"].bitcast(fp32)[:DET, :DET],
                        )
                    st = ob.tile([D, 3 * DET], fp32, name="st", tag="st")
                    nc.scalar.copy(out=st, in_=pot)
                    ra, rb = r0 + t * D, r0 + (t + 1) * D
                    nc.sync.dma_start(out=odet[ra:rb, :], in_=st[:, 0:DET])
                    nc.sync.dma_start(out=ol1[ra:rb, :], in_=st[:, DET:2 * DET])
                    nc.sync.dma_start(out=ol2[ra:rb, :], in_=st[:, 2 * DET:3 * DET])

    nc.finalize()
    return nc


def _get_nc(rows, layout, tot, elu_mid_engine="vector"):
    key = (rows, elu_mid_engine)
    if key not in _BUILD_CACHE:
        _BUILD_CACHE[key] = _build_nc(rows, layout, tot, elu_mid_engine)
    return _BUILD_CACHE[key]


def kernel(sampled_state, actions, params, _trace=False, _ncores=NCORES):
    from concourse.bass_utils import run_bass_kernel_spmd

    x = _np32(sampled_state)
    a = _np32(actions)
    n = x.shape[0]
    rows = n // _ncores
    assert rows % F == 0

    tensors = _prep_tensors(params)
    layout, tot = _pack_layout(tensors)
    wpack = _pack(tensors, layout, tot)
    nc = _get_nc(rows, layout, tot)

    in_maps = []
    for c in range(_ncores):
        in_maps.append({
            "x": np.ascontiguousarray(x[c * rows:(c + 1) * rows]),
            "act": np.ascontiguousarray(a[c * rows:(c + 1) * rows]),
            "wpack": wpack,
        })

    res = run_bass_kernel_spmd(
        nc, in_maps, core_ids=list(range(_ncores)), trace=_trace,
    )
    results = res.results
    det = np.concatenate([r["odet"] for r in results], axis=0)
    l1 = np.concatenate([r["ol1"] for r in results], axis=0)
    l2 = np.concatenate([r["ol2"] for r in results], axis=0)
    if _trace:
        return (det, l1, l2), res
    return (det, l1, l2)
